# revision 15
# baseline (speedup 1.0000x reference)
"""Bilateral filter (d=7, sc=0.1, ss=3.0) on 8 Trainium2 cores — slab variant.

Same Gaussian-sum decomposition as kernel.py (K=10 LS-fitted), but the
row-major stages (DErf eval B_k, product P_k = B_k*x) run once per 2-band
slab (242 cols) instead of per 128-col band: ~18% less ACT work, ~11% less
DVE work, half the row-stage instructions.  P_k for all k is kept in SBUF
(12-deep pool) and the per-band col-major tails slice it as matmul lhsT.
"""
import json
import os
import numpy as np

D = 7
SIGMA_COLOR = 0.1
SIGMA_SPACE = 3.0

N_CORES = 8
PLANES = 6
H = W = 768
RPAD = 812
CPAD = 814
NB = 7                  # col bands
BW = 114
BIN = 128
NT = 7
TH = 114
THT = NT * TH           # 798
SLAB_NB = 2             # bands per slab
K_IMG = 10
A_BASIS = 100.0
Q_MARGIN = 0.0

_CACHE = {}


def _grid(k_img=K_IMG, margin=Q_MARGIN):
    q = np.linspace(-margin, 1.0 + margin, k_img)
    m = np.linspace(0.0, 1.0, 4001)
    phi = np.exp(-2 * A_BASIS * (m[:, None] - q[None, :]) ** 2)
    c = np.linalg.solve(phi.T @ phi, phi.T @ np.ones(len(m)))
    wgts = [float(ck * np.pi / 4.0) for ck in c]
    return q, wgts


def _g1n():
    offs = np.arange(-D, D + 1)
    g = np.exp(-0.5 * offs ** 2 / SIGMA_SPACE ** 2)
    return (g / g.sum()).astype(np.float64)


def _sw00():
    g = np.exp(-0.5 * np.arange(-D, D + 1) ** 2 / SIGMA_SPACE ** 2)
    sw = np.outer(g, g)
    return float((sw / sw.sum())[D, D])


def _consts():
    g1n = _g1n()
    bband = np.zeros((128, TH), np.float16)
    for ri in range(128):
        for ro in range(TH):
            d = ri - ro
            if 0 <= d <= 2 * D:
                bband[ri, ro] = g1n[d]
    cband = np.zeros((BIN, BIN), np.float16)
    for ci in range(BIN):
        for p in range(BIN):
            dd = ci - (p - D)
            if D <= p < D + BW and 0 <= dd <= 2 * D:
                cband[ci, p] = g1n[dd]
    ident16 = np.eye(128, dtype=np.float16)
    idneg = (-_sw00() * np.eye(BIN)).astype(np.float16)
    return bband, cband, ident16, idneg


DEFAULTS = dict(
    k_img=K_IMG,
    slab_nb=SLAB_NB,
    tk_split=0,
    tk_gp=0,
    zk_act=10,
    evict_dve=5,
    p_gpsimd=0,
    copies_gp=True,
    pk_bufs=12,
    sb_bufs=7, ph_bufs=2, pz_bufs=1, pa_bufs=1,
)


def build(reps=1, **overrides):
    cfg = dict(DEFAULTS)
    cfg.update(json.loads(os.environ.get("KERNEL_BUILD_KWARGS", "{}")))
    cfg.update(overrides)
    k_img = cfg["k_img"]
    slab_nb = cfg["slab_nb"]
    slabw = 128 + (slab_nb - 1) * BW
    tk_split, zk_act = cfg["tk_split"], cfg["zk_act"]
    tk_gp = cfg["tk_gp"]
    evict_dve = cfg["evict_dve"]
    p_gpsimd, copies_gp = cfg["p_gpsimd"], cfg["copies_gp"]
    pk_bufs, sb_bufs = cfg["pk_bufs"], cfg["sb_bufs"]
    ph_bufs, pz_bufs, pa_bufs = cfg["ph_bufs"], cfg["pz_bufs"], cfg["pa_bufs"]

    import concourse.tile as tile
    import concourse.bass as bass
    from concourse import bacc, mybir

    f32 = mybir.dt.float32
    fp16 = mybir.dt.float16
    AF = mybir.ActivationFunctionType
    ALU = mybir.AluOpType

    q, wgts = _grid(k_img)
    nc = bacc.Bacc("TRN2", target_bir_lowering=False, debug=False,
                   num_devices=N_CORES)
    xp = nc.dram_tensor("xp", [PLANES, RPAD, CPAD], f32, kind="ExternalInput")
    out = nc.dram_tensor("out", [PLANES, H, W], f32, kind="ExternalOutput")
    bband_d = nc.dram_tensor("bband", [128, TH], fp16, kind="ExternalInput")
    cband_d = nc.dram_tensor("cband", [BIN, BIN], fp16, kind="ExternalInput")
    ident16_d = nc.dram_tensor("ident16", [128, 128], fp16, kind="ExternalInput")
    idneg_d = nc.dram_tensor("idneg", [BIN, BIN], fp16, kind="ExternalInput")

    with tile.TileContext(nc) as tc:
        with (
            tc.tile_pool(name="consts", bufs=1) as consts,
            tc.tile_pool(name="xf_pool", bufs=2) as xf_pool,
            tc.tile_pool(name="x16_pool", bufs=2) as x16_pool,
            tc.tile_pool(name="xt_pool", bufs=2) as xt_pool,
            tc.tile_pool(name="b_pool", bufs=3) as b_pool,
            tc.tile_pool(name="p_pool", bufs=pk_bufs) as p_pool,
            tc.tile_pool(name="y_pool", bufs=sb_bufs) as y_pool,
            tc.tile_pool(name="bt_pool", bufs=sb_bufs) as bt_pool,
            tc.tile_pool(name="z_pool", bufs=sb_bufs) as z_pool,
            tc.tile_pool(name="t_pool", bufs=sb_bufs) as t_pool,
            tc.tile_pool(name="of_pool", bufs=2) as of_pool,
            tc.tile_pool(name="ph_pool", bufs=ph_bufs, space="PSUM") as ph_pool,
            tc.tile_pool(name="pz_pool", bufs=pz_bufs, space="PSUM") as pz_pool,
            tc.tile_pool(name="pa_pool", bufs=pa_bufs, space="PSUM") as pa_pool,
        ):
            bb = consts.tile([128, TH], fp16)
            nc.sync.dma_start(out=bb[:], in_=bband_d.ap())
            cb = consts.tile([BIN, BIN], fp16)
            nc.sync.dma_start(out=cb[:], in_=cband_d.ap())
            id16 = consts.tile([128, 128], fp16)
            nc.sync.dma_start(out=id16[:], in_=ident16_d.ap())
            idn = consts.tile([BIN, BIN], fp16)
            nc.sync.dma_start(out=idn[:], in_=idneg_d.ap())
            biases = consts.tile([128, k_img], f32)
            for k in range(k_img):
                nc.gpsimd.memset(biases[:, k:k + 1], float(-10.0 * q[k]))

            def band_tail(plane, band, x16s, pks, off):
                """Col-major tail for one 128-in-col band; row-major inputs
                come from the slab tiles at column offset `off`."""
                c0 = BW * band
                ncols = min(BW, W - c0)
                # x^T via matmul transpose
                pxt = ph_pool.tile([BIN, NT, 128], f32, tag="ph")
                for t in range(NT):
                    nc.tensor.matmul(pxt[:, t, 0:128],
                                     x16s[:, t, off:off + 128], id16[:],
                                     start=True, stop=True)
                xt16 = xt_pool.tile([BIN, THT], fp16, tag="xt16")
                nc.vector.tensor_copy(xt16[:], pxt[:, :, D:D + TH])

                pacc = pa_pool.tile([BIN, H], f32, tag="pa")
                nmm = [0]
                LAST = 2 * (k_img + 1)

                def acc_mm(lhsT, rhs):
                    for (a, b) in ((0, 512), (512, H)):
                        nc.tensor.matmul(pacc[:, a:b], lhsT, rhs[:, a:b],
                                         start=(nmm[0] < 2),
                                         stop=(nmm[0] >= LAST - 2))
                        nmm[0] += 1

                for k in range(k_img):
                    # H-conv + transpose fused from the kept slab P_k
                    ph = ph_pool.tile([BIN, NT, 128], f32, tag="ph")
                    for t in range(NT):
                        nc.tensor.matmul(ph[:, t, 0:TH],
                                         pks[k][:, t, off:off + 128], bb[:],
                                         start=True, stop=True)
                    yk = y_pool.tile([BIN, THT], fp16, tag="yk")
                    yeng = nc.vector if (k % k_img) < evict_dve else nc.scalar
                    if yeng is nc.vector:
                        nc.vector.tensor_copy(yk[:], ph[:, :, 0:TH])
                    else:
                        nc.scalar.copy(yk[:], ph[:, :, 0:TH])
                    pz = pz_pool.tile([BIN, H], f32, tag="pz")
                    nc.tensor.matmul(pz[:, 0:512], cb[:], yk[:, 0:512],
                                     start=True, stop=True)
                    nc.tensor.matmul(pz[:, 512:H], cb[:], yk[:, 512:H],
                                     start=True, stop=True)
                    btk = bt_pool.tile([BIN, H], fp16, tag="btk")
                    nc.scalar.activation(btk[:], xt16[:, 0:H],
                                         AF.Derivative_Erf, scale=10.0,
                                         bias=biases[0:BIN, k:k + 1])
                    tk = t_pool.tile([BIN, H], fp16, tag="tk")
                    if tk_split:
                        zk = z_pool.tile([BIN, H], fp16, tag="zk")
                        if (k % k_img) < zk_act:
                            nc.scalar.mul(zk[:], pz[:], wgts[k])
                        else:
                            nc.vector.tensor_scalar_mul(zk[:], pz[:], wgts[k])
                        teng = nc.gpsimd if (k % k_img) < tk_gp else nc.vector
                        teng.tensor_tensor(tk[:], btk[:], zk[:], ALU.mult)
                    else:
                        nc.vector.scalar_tensor_tensor(tk[:], btk[:], wgts[k],
                                                       pz[:], ALU.mult,
                                                       ALU.mult)
                    acc_mm(id16[0:BIN, 0:BIN], tk)
                acc_mm(idn[:], xt16[:, 0:H])
                of = of_pool.tile([BIN, H], fp16, tag="of")
                nc.vector.tensor_copy(of[:], pacc[:])
                pb = ph_pool.tile([BIN, NT, 128], f32, tag="ph")
                for t in range(6):
                    nc.tensor.matmul(pb[0:128, t, 0:ncols],
                                     of[:, 128 * t:128 * (t + 1)],
                                     id16[0:BIN, D:D + ncols],
                                     start=True, stop=True)
                ob = of_pool.tile([128, 6, BW], f32, tag="ob")
                oeng = nc.scalar if (plane + band) % 2 == 0 else nc.vector
                if oeng is nc.scalar:
                    nc.scalar.copy(ob[:, :, 0:ncols], pb[:, 0:6, 0:ncols])
                else:
                    nc.vector.tensor_copy(ob[:, :, 0:ncols], pb[:, 0:6, 0:ncols])
                obase = plane * H * W + c0
                nc.sync.dma_start(
                    out=bass.AP(tensor=out, offset=obase,
                                ap=[[W, 128], [128 * W, 6], [1, ncols]]),
                    in_=ob[:, :, 0:ncols])

            def slab_unit(plane, s0):
                bands = list(range(s0, min(s0 + slab_nb, NB)))
                C0 = BW * s0
                wp = 128 + (len(bands) - 1) * BW
                xfs = xf_pool.tile([128, NT, slabw], f32, tag="xf")
                base = plane * RPAD * CPAD + C0
                nc.sync.dma_start(
                    out=xfs[:, :, 0:wp],
                    in_=bass.AP(tensor=xp, offset=base,
                                ap=[[CPAD, 128], [TH * CPAD, NT], [1, wp]]))
                x16s = x16_pool.tile([128, NT, slabw], fp16, tag="x16")
                (nc.gpsimd if copies_gp else nc.vector).tensor_copy(
                    x16s[:, :, 0:wp], xfs[:, :, 0:wp])
                pks = []
                for k in range(k_img):
                    bks = b_pool.tile([128, NT, slabw], fp16, tag="bk")
                    nc.scalar.activation(bks[:, :, 0:wp], xfs[:, :, 0:wp],
                                         AF.Derivative_Erf, scale=10.0,
                                         bias=biases[:, k:k + 1])
                    pk = p_pool.tile([128, NT, slabw], fp16, tag="pk")
                    peng = nc.gpsimd if (k % k_img) < p_gpsimd else nc.vector
                    peng.tensor_tensor(pk[:, :, 0:wp], bks[:, :, 0:wp],
                                       x16s[:, :, 0:wp], ALU.mult)
                    pks.append(pk)
                for b in bands:
                    band_tail(plane, b, x16s, pks, BW * (b - s0))

            def body(_iv=None):
                for plane in range(PLANES):
                    for s0 in range(0, NB, slab_nb):
                        slab_unit(plane, s0)

            if reps == 1:
                body()
            else:
                with tc.For_i(0, reps, 1) as _i:
                    body(_i)
    nc.compile()
    return nc


def _prepare_inputs(x):
    planes = np.ascontiguousarray(x.reshape(N_CORES, PLANES, H, W))
    bband, cband, ident16, idneg = _consts()
    in_maps = []
    for c in range(N_CORES):
        xpad = np.pad(planes[c], ((0, 0), (D, RPAD - H - D), (D, CPAD - W - D)),
                      mode="reflect")
        in_maps.append({"xp": np.ascontiguousarray(xpad),
                        "bband": bband, "cband": cband, "ident16": ident16,
                        "idneg": idneg})
    return in_maps


def _gather_outputs(results):
    outs = [results[c]["out"] for c in range(N_CORES)]
    return np.stack(outs).reshape(16, 3, H, W).astype(np.float32)


def kernel(x):
    from concourse.bass_utils import run_bass_kernel_spmd

    x = np.asarray(x, dtype=np.float32)
    if "nc" not in _CACHE:
        _CACHE["nc"] = build(reps=1)
    in_maps = _prepare_inputs(x)
    res = run_bass_kernel_spmd(_CACHE["nc"], in_maps,
                               core_ids=list(range(N_CORES)))
    return _gather_outputs(res.results)


# revision 21
# speedup vs baseline: 1.1060x; 1.1060x over previous
"""Bilateral filter (d=7, sc=0.1, ss=3.0) on 8 Trainium2 cores — slab variant.

Same Gaussian-sum decomposition as kernel.py (K=10 LS-fitted), but the
row-major stages (DErf eval B_k, product P_k = B_k*x) run once per 2-band
slab (242 cols) instead of per 128-col band: ~18% less ACT work, ~11% less
DVE work, half the row-stage instructions.  P_k for all k is kept in SBUF
(12-deep pool) and the per-band col-major tails slice it as matmul lhsT.
"""
import json
import os
import numpy as np

D = 7
SIGMA_COLOR = 0.1
SIGMA_SPACE = 3.0

N_CORES = 8
PLANES = 6
H = W = 768
RPAD = 812
CPAD = 814
NB = 7                  # col bands
BW = 114
BIN = 128
NT = 7
TH = 114
THT = NT * TH           # 798
SLAB_NB = 2             # bands per slab
K_IMG = 10
A_BASIS = 100.0
Q_MARGIN = 0.0

_CACHE = {}


def _grid(k_img=K_IMG, margin=Q_MARGIN):
    q = np.linspace(-margin, 1.0 + margin, k_img)
    m = np.linspace(0.0, 1.0, 4001)
    phi = np.exp(-2 * A_BASIS * (m[:, None] - q[None, :]) ** 2)
    c = np.linalg.solve(phi.T @ phi, phi.T @ np.ones(len(m)))
    wgts = [float(ck * np.pi / 4.0) for ck in c]
    return q, wgts


def _g1n():
    offs = np.arange(-D, D + 1)
    g = np.exp(-0.5 * offs ** 2 / SIGMA_SPACE ** 2)
    return (g / g.sum()).astype(np.float64)


def _sw00():
    g = np.exp(-0.5 * np.arange(-D, D + 1) ** 2 / SIGMA_SPACE ** 2)
    sw = np.outer(g, g)
    return float((sw / sw.sum())[D, D])


def _consts():
    g1n = _g1n()
    bband = np.zeros((128, TH), np.float16)
    for ri in range(128):
        for ro in range(TH):
            d = ri - ro
            if 0 <= d <= 2 * D:
                bband[ri, ro] = g1n[d]
    cband = np.zeros((BIN, BIN), np.float16)
    for ci in range(BIN):
        for p in range(BIN):
            dd = ci - (p - D)
            if D <= p < D + BW and 0 <= dd <= 2 * D:
                cband[ci, p] = g1n[dd]
    ident16 = np.eye(128, dtype=np.float16)
    idneg = (-_sw00() * np.eye(BIN)).astype(np.float16)
    return bband, cband, ident16, idneg


DEFAULTS = dict(
    k_img=K_IMG,
    slab_nb=SLAB_NB,
    tk_split=0,
    tk_gp=0,
    zk_act=10,
    evict_dve=3,
    xt_act=0,        # (unused in V7: xt16 comes via DMA transpose)
    of_act=0,        # of evict on ScalarE instead of DVE
    btk_dma=0,       # dead: SBUF->SBUF xbar transpose is wrong+slow; keep 0
    p_gpsimd=0,
    copies_gp=True,
    pk_bufs=12,
    sb_bufs=7, ph_bufs=2, pz_bufs=1, pa_bufs=1,
)


def build(reps=1, **overrides):
    cfg = dict(DEFAULTS)
    cfg.update(json.loads(os.environ.get("KERNEL_BUILD_KWARGS", "{}")))
    cfg.update(overrides)
    k_img = cfg["k_img"]
    slab_nb = cfg["slab_nb"]
    slabw = 128 + (slab_nb - 1) * BW
    tk_split, zk_act = cfg["tk_split"], cfg["zk_act"]
    tk_gp = cfg["tk_gp"]
    evict_dve = cfg["evict_dve"]
    xt_act, of_act = cfg["xt_act"], cfg["of_act"]
    btk_dma = cfg["btk_dma"]
    p_gpsimd, copies_gp = cfg["p_gpsimd"], cfg["copies_gp"]
    pk_bufs, sb_bufs = cfg["pk_bufs"], cfg["sb_bufs"]
    ph_bufs, pz_bufs, pa_bufs = cfg["ph_bufs"], cfg["pz_bufs"], cfg["pa_bufs"]

    import concourse.tile as tile
    import concourse.bass as bass
    from concourse import bacc, mybir

    f32 = mybir.dt.float32
    fp16 = mybir.dt.float16
    AF = mybir.ActivationFunctionType
    ALU = mybir.AluOpType

    q, wgts = _grid(k_img)
    nc = bacc.Bacc("TRN2", target_bir_lowering=False, debug=False,
                   num_devices=N_CORES)
    xp = nc.dram_tensor("xp", [PLANES, RPAD, CPAD], fp16, kind="ExternalInput")
    out = nc.dram_tensor("out", [PLANES, H, W], f32, kind="ExternalOutput")
    bband_d = nc.dram_tensor("bband", [128, TH], fp16, kind="ExternalInput")
    cband_d = nc.dram_tensor("cband", [BIN, BIN], fp16, kind="ExternalInput")
    ident16_d = nc.dram_tensor("ident16", [128, 128], fp16, kind="ExternalInput")
    idneg_d = nc.dram_tensor("idneg", [BIN, BIN], fp16, kind="ExternalInput")

    with tile.TileContext(nc) as tc:
        with (
            tc.tile_pool(name="consts", bufs=1) as consts,
            tc.tile_pool(name="xf_pool", bufs=2) as xf_pool,
            tc.tile_pool(name="x16_pool", bufs=2) as x16_pool,
            tc.tile_pool(name="xt_pool", bufs=2) as xt_pool,
            tc.tile_pool(name="b_pool",
                         bufs=(pk_bufs if btk_dma else 3)) as b_pool,
            tc.tile_pool(name="p_pool", bufs=pk_bufs) as p_pool,
            tc.tile_pool(name="y_pool", bufs=sb_bufs) as y_pool,
            tc.tile_pool(name="bt_pool", bufs=sb_bufs) as bt_pool,
            tc.tile_pool(name="z_pool", bufs=sb_bufs) as z_pool,
            tc.tile_pool(name="t_pool", bufs=sb_bufs) as t_pool,
            tc.tile_pool(name="of_pool", bufs=2) as of_pool,
            tc.tile_pool(name="ph_pool", bufs=ph_bufs, space="PSUM") as ph_pool,
            tc.tile_pool(name="pz_pool", bufs=pz_bufs, space="PSUM") as pz_pool,
            tc.tile_pool(name="pa_pool", bufs=pa_bufs, space="PSUM") as pa_pool,
        ):
            bb = consts.tile([128, TH], fp16)
            nc.sync.dma_start(out=bb[:], in_=bband_d.ap())
            cb = consts.tile([BIN, BIN], fp16)
            nc.sync.dma_start(out=cb[:], in_=cband_d.ap())
            id16 = consts.tile([128, 128], fp16)
            nc.sync.dma_start(out=id16[:], in_=ident16_d.ap())
            idn = consts.tile([BIN, BIN], fp16)
            nc.sync.dma_start(out=idn[:], in_=idneg_d.ap())
            biases = consts.tile([128, k_img], f32)
            for k in range(k_img):
                nc.gpsimd.memset(biases[:, k:k + 1], float(-10.0 * q[k]))

            def band_tail(plane, band, x16s, pks, bkl, off):
                """Col-major tail for one 128-in-col band; row-major inputs
                come from the slab tiles at column offset `off`."""
                c0 = BW * band
                ncols = min(BW, W - c0)
                # x^T for this band straight from DRAM via the xbar transpose
                xt16 = xt_pool.tile([BIN, H], fp16, tag="xt16")
                nc.sync.dma_start(
                    out=xt16[:],
                    in_=bass.AP(tensor=xp,
                                offset=plane * RPAD * CPAD + D * CPAD + c0,
                                ap=[[CPAD, H], [1, 128]]),
                    transpose=True)

                pacc = pa_pool.tile([BIN, H], f32, tag="pa")
                nmm = [0]
                LAST = 2 * (k_img + 1)

                def acc_mm(lhsT, rhs):
                    for (a, b) in ((0, 512), (512, H)):
                        nc.tensor.matmul(pacc[:, a:b], lhsT, rhs[:, a:b],
                                         start=(nmm[0] < 2),
                                         stop=(nmm[0] >= LAST - 2))
                        nmm[0] += 1

                for k in range(k_img):
                    # H-conv + transpose fused from the kept slab P_k
                    ph = ph_pool.tile([BIN, NT, 128], f32, tag="ph")
                    for t in range(NT):
                        nc.tensor.matmul(ph[:, t, 0:TH],
                                         pks[k][:, t, off:off + 128], bb[:],
                                         start=True, stop=True)
                    yk = y_pool.tile([BIN, THT], fp16, tag="yk")
                    yeng = nc.vector if (k % k_img) < evict_dve else nc.scalar
                    if yeng is nc.vector:
                        nc.vector.tensor_copy(yk[:], ph[:, :, 0:TH])
                    else:
                        nc.scalar.copy(yk[:], ph[:, :, 0:TH])
                    pz = pz_pool.tile([BIN, H], f32, tag="pz")
                    nc.tensor.matmul(pz[:, 0:512], cb[:], yk[:, 0:512],
                                     start=True, stop=True)
                    nc.tensor.matmul(pz[:, 512:H], cb[:], yk[:, 512:H],
                                     start=True, stop=True)
                    assert btk_dma == 0
                    btp = bt_pool.tile([BIN, H], fp16, tag="btk")
                    nc.scalar.activation(btp[:], xt16[:],
                                         AF.Derivative_Erf, scale=10.0,
                                         bias=biases[0:BIN, k:k + 1])
                    btk = btp[:]
                    tk = t_pool.tile([BIN, H], fp16, tag="tk")
                    if tk_split:
                        zk = z_pool.tile([BIN, H], fp16, tag="zk")
                        if (k % k_img) < zk_act:
                            nc.scalar.mul(zk[:], pz[:], wgts[k])
                        else:
                            nc.vector.tensor_scalar_mul(zk[:], pz[:], wgts[k])
                        teng = nc.gpsimd if (k % k_img) < tk_gp else nc.vector
                        teng.tensor_tensor(tk[:], btk, zk[:], ALU.mult)
                    else:
                        nc.vector.scalar_tensor_tensor(tk[:], btk, wgts[k],
                                                       pz[:], ALU.mult,
                                                       ALU.mult)
                    acc_mm(id16[0:BIN, 0:BIN], tk)
                acc_mm(idn[:], xt16[:])
                of = of_pool.tile([BIN, H], fp16, tag="of")
                if of_act:
                    nc.scalar.copy(of[:], pacc[:])
                else:
                    nc.vector.tensor_copy(of[:], pacc[:])
                pb = ph_pool.tile([BIN, NT, 128], f32, tag="ph")
                for t in range(6):
                    nc.tensor.matmul(pb[0:128, t, 0:ncols],
                                     of[:, 128 * t:128 * (t + 1)],
                                     id16[0:BIN, D:D + ncols],
                                     start=True, stop=True)
                ob = of_pool.tile([128, 6, BW], f32, tag="ob")
                oeng = nc.scalar if (plane + band) % 2 == 0 else nc.vector
                if oeng is nc.scalar:
                    nc.scalar.copy(ob[:, :, 0:ncols], pb[:, 0:6, 0:ncols])
                else:
                    nc.vector.tensor_copy(ob[:, :, 0:ncols], pb[:, 0:6, 0:ncols])
                obase = plane * H * W + c0
                nc.sync.dma_start(
                    out=bass.AP(tensor=out, offset=obase,
                                ap=[[W, 128], [128 * W, 6], [1, ncols]]),
                    in_=ob[:, :, 0:ncols])

            def slab_unit(plane, s0):
                bands = list(range(s0, min(s0 + slab_nb, NB)))
                C0 = BW * s0
                wp = 128 + (len(bands) - 1) * BW
                base = plane * RPAD * CPAD + C0
                x16s = x16_pool.tile([128, NT, slabw], fp16, tag="x16")
                nc.sync.dma_start(
                    out=x16s[:, :, 0:wp],
                    in_=bass.AP(tensor=xp, offset=base,
                                ap=[[CPAD, 128], [TH * CPAD, NT], [1, wp]]))
                pks = []
                bkl = []
                for k in range(k_img):
                    bks = b_pool.tile([128, NT, slabw], fp16, tag="bk")
                    nc.scalar.activation(bks[:, :, 0:wp], x16s[:, :, 0:wp],
                                         AF.Derivative_Erf, scale=10.0,
                                         bias=biases[:, k:k + 1])
                    pk = p_pool.tile([128, NT, slabw], fp16, tag="pk")
                    peng = nc.gpsimd if (k % k_img) < p_gpsimd else nc.vector
                    peng.tensor_tensor(pk[:, :, 0:wp], bks[:, :, 0:wp],
                                       x16s[:, :, 0:wp], ALU.mult)
                    pks.append(pk)
                    bkl.append(bks)
                for b in bands:
                    band_tail(plane, b, x16s, pks, bkl, BW * (b - s0))

            def body(_iv=None):
                for plane in range(PLANES):
                    for s0 in range(0, NB, slab_nb):
                        slab_unit(plane, s0)

            if reps == 1:
                body()
            else:
                with tc.For_i(0, reps, 1) as _i:
                    body(_i)
    nc.compile()
    return nc


def _prepare_inputs(x):
    planes = np.ascontiguousarray(x.reshape(N_CORES, PLANES, H, W))
    bband, cband, ident16, idneg = _consts()
    in_maps = []
    for c in range(N_CORES):
        xpad = np.pad(planes[c], ((0, 0), (D, RPAD - H - D), (D, CPAD - W - D)),
                      mode="reflect").astype(np.float16)
        in_maps.append({"xp": np.ascontiguousarray(xpad),
                        "bband": bband, "cband": cband, "ident16": ident16,
                        "idneg": idneg})
    return in_maps


def _gather_outputs(results):
    outs = [results[c]["out"] for c in range(N_CORES)]
    return np.stack(outs).reshape(16, 3, H, W).astype(np.float32)


def kernel(x):
    from concourse.bass_utils import run_bass_kernel_spmd

    x = np.asarray(x, dtype=np.float32)
    if "nc" not in _CACHE:
        _CACHE["nc"] = build(reps=1)
    in_maps = _prepare_inputs(x)
    res = run_bass_kernel_spmd(_CACHE["nc"], in_maps,
                               core_ids=list(range(N_CORES)))
    return _gather_outputs(res.results)


# revision 22
# speedup vs baseline: 1.1213x; 1.0138x over previous
"""Bilateral filter (d=7, sc=0.1, ss=3.0) on 8 Trainium2 cores — slab variant.

Same Gaussian-sum decomposition as kernel.py (K=10 LS-fitted), but the
row-major stages (DErf eval B_k, product P_k = B_k*x) run once per 2-band
slab (242 cols) instead of per 128-col band: ~18% less ACT work, ~11% less
DVE work, half the row-stage instructions.  P_k for all k is kept in SBUF
(12-deep pool) and the per-band col-major tails slice it as matmul lhsT.
"""
import json
import os
import numpy as np

D = 7
SIGMA_COLOR = 0.1
SIGMA_SPACE = 3.0

N_CORES = 8
PLANES = 6
H = W = 768
RPAD = 812
CPAD = 814
NB = 7                  # col bands
BW = 114
BIN = 128
NT = 7
TH = 114
THT = NT * TH           # 798
SLAB_NB = 2             # bands per slab
K_IMG = 10
A_BASIS = 100.0
Q_MARGIN = 0.0

_CACHE = {}


def _grid(k_img=K_IMG, margin=Q_MARGIN):
    q = np.linspace(-margin, 1.0 + margin, k_img)
    m = np.linspace(0.0, 1.0, 4001)
    phi = np.exp(-2 * A_BASIS * (m[:, None] - q[None, :]) ** 2)
    c = np.linalg.solve(phi.T @ phi, phi.T @ np.ones(len(m)))
    wgts = [float(ck * np.pi / 4.0) for ck in c]
    return q, wgts


def _g1n():
    offs = np.arange(-D, D + 1)
    g = np.exp(-0.5 * offs ** 2 / SIGMA_SPACE ** 2)
    return (g / g.sum()).astype(np.float64)


def _sw00():
    g = np.exp(-0.5 * np.arange(-D, D + 1) ** 2 / SIGMA_SPACE ** 2)
    sw = np.outer(g, g)
    return float((sw / sw.sum())[D, D])


def _consts():
    g1n = _g1n()
    bband = np.zeros((128, TH), np.float16)
    for ri in range(128):
        for ro in range(TH):
            d = ri - ro
            if 0 <= d <= 2 * D:
                bband[ri, ro] = g1n[d]
    cband = np.zeros((BIN, BIN), np.float16)
    for ci in range(BIN):
        for p in range(BIN):
            dd = ci - (p - D)
            if D <= p < D + BW and 0 <= dd <= 2 * D:
                cband[ci, p] = g1n[dd]
    ident16 = np.eye(128, dtype=np.float16)
    idneg = (-_sw00() * np.eye(BIN)).astype(np.float16)
    return bband, cband, ident16, idneg


DEFAULTS = dict(
    k_img=K_IMG,
    slab_nb=SLAB_NB,
    tk_split=0,
    tk_gp=0,
    zk_act=10,
    evict_dve=3,
    xt_act=0,        # (unused in V7: xt16 comes via DMA transpose)
    of_act=0,        # of evict on ScalarE instead of DVE
    btk_dma=0,       # dead: SBUF->SBUF xbar transpose is wrong+slow; keep 0
    hoist_dma=0,     # issue input DMAs at high scheduler priority
    p_gpsimd=0,
    copies_gp=True,
    pk_bufs=12,
    sb_bufs=7, ph_bufs=2, pz_bufs=1, pa_bufs=1,
)


def build(reps=1, **overrides):
    cfg = dict(DEFAULTS)
    cfg.update(json.loads(os.environ.get("KERNEL_BUILD_KWARGS", "{}")))
    cfg.update(overrides)
    k_img = cfg["k_img"]
    slab_nb = cfg["slab_nb"]
    slabw = 128 + (slab_nb - 1) * BW
    tk_split, zk_act = cfg["tk_split"], cfg["zk_act"]
    tk_gp = cfg["tk_gp"]
    evict_dve = cfg["evict_dve"]
    xt_act, of_act = cfg["xt_act"], cfg["of_act"]
    btk_dma = cfg["btk_dma"]
    hoist_dma = cfg["hoist_dma"]
    p_gpsimd, copies_gp = cfg["p_gpsimd"], cfg["copies_gp"]
    pk_bufs, sb_bufs = cfg["pk_bufs"], cfg["sb_bufs"]
    ph_bufs, pz_bufs, pa_bufs = cfg["ph_bufs"], cfg["pz_bufs"], cfg["pa_bufs"]

    import concourse.tile as tile
    import concourse.bass as bass
    from concourse import bacc, mybir

    f32 = mybir.dt.float32
    fp16 = mybir.dt.float16
    AF = mybir.ActivationFunctionType
    ALU = mybir.AluOpType

    q, wgts = _grid(k_img)
    nc = bacc.Bacc("TRN2", target_bir_lowering=False, debug=False,
                   num_devices=N_CORES)
    xp = nc.dram_tensor("xp", [PLANES, RPAD, CPAD], fp16, kind="ExternalInput")
    out = nc.dram_tensor("out", [PLANES, H, W], f32, kind="ExternalOutput")
    bband_d = nc.dram_tensor("bband", [128, TH], fp16, kind="ExternalInput")
    cband_d = nc.dram_tensor("cband", [BIN, BIN], fp16, kind="ExternalInput")
    ident16_d = nc.dram_tensor("ident16", [128, 128], fp16, kind="ExternalInput")
    idneg_d = nc.dram_tensor("idneg", [BIN, BIN], fp16, kind="ExternalInput")

    with tile.TileContext(nc) as tc:
        with (
            tc.tile_pool(name="consts", bufs=1) as consts,
            tc.tile_pool(name="xf_pool", bufs=2) as xf_pool,
            tc.tile_pool(name="x16_pool", bufs=2) as x16_pool,
            tc.tile_pool(name="xt_pool", bufs=2) as xt_pool,
            tc.tile_pool(name="b_pool",
                         bufs=(pk_bufs if btk_dma else 3)) as b_pool,
            tc.tile_pool(name="p_pool", bufs=pk_bufs) as p_pool,
            tc.tile_pool(name="y_pool", bufs=sb_bufs) as y_pool,
            tc.tile_pool(name="bt_pool", bufs=sb_bufs) as bt_pool,
            tc.tile_pool(name="z_pool", bufs=sb_bufs) as z_pool,
            tc.tile_pool(name="t_pool", bufs=sb_bufs) as t_pool,
            tc.tile_pool(name="of_pool", bufs=2) as of_pool,
            tc.tile_pool(name="ph_pool", bufs=ph_bufs, space="PSUM") as ph_pool,
            tc.tile_pool(name="pz_pool", bufs=pz_bufs, space="PSUM") as pz_pool,
            tc.tile_pool(name="pa_pool", bufs=pa_bufs, space="PSUM") as pa_pool,
        ):
            bb = consts.tile([128, TH], fp16)
            nc.sync.dma_start(out=bb[:], in_=bband_d.ap())
            cb = consts.tile([BIN, BIN], fp16)
            nc.sync.dma_start(out=cb[:], in_=cband_d.ap())
            id16 = consts.tile([128, 128], fp16)
            nc.sync.dma_start(out=id16[:], in_=ident16_d.ap())
            idn = consts.tile([BIN, BIN], fp16)
            nc.sync.dma_start(out=idn[:], in_=idneg_d.ap())
            biases = consts.tile([128, k_img], f32)
            for k in range(k_img):
                nc.gpsimd.memset(biases[:, k:k + 1], float(-10.0 * q[k]))

            def band_tail(plane, band, x16s, pks, bkl, off):
                """Col-major tail for one 128-in-col band; row-major inputs
                come from the slab tiles at column offset `off`."""
                c0 = BW * band
                ncols = min(BW, W - c0)
                # x^T for this band straight from DRAM via the xbar transpose
                xt16 = xt_pool.tile([BIN, H], fp16, tag="xt16")
                from contextlib import nullcontext
                with (tc.high_priority() if hoist_dma else nullcontext()):
                    nc.sync.dma_start(
                        out=xt16[:],
                        in_=bass.AP(tensor=xp,
                                    offset=plane * RPAD * CPAD + D * CPAD + c0,
                                    ap=[[CPAD, H], [1, 128]]),
                        transpose=True)

                pacc = pa_pool.tile([BIN, H], f32, tag="pa")
                nmm = [0]
                LAST = 2 * (k_img + 1)

                def acc_mm(lhsT, rhs):
                    for (a, b) in ((0, 512), (512, H)):
                        nc.tensor.matmul(pacc[:, a:b], lhsT, rhs[:, a:b],
                                         start=(nmm[0] < 2),
                                         stop=(nmm[0] >= LAST - 2))
                        nmm[0] += 1

                for k in range(k_img):
                    # H-conv + transpose fused from the kept slab P_k
                    ph = ph_pool.tile([BIN, NT, 128], f32, tag="ph")
                    for t in range(NT):
                        nc.tensor.matmul(ph[:, t, 0:TH],
                                         pks[k][:, t, off:off + 128], bb[:],
                                         start=True, stop=True)
                    yk = y_pool.tile([BIN, THT], fp16, tag="yk")
                    yeng = nc.vector if (k % k_img) < evict_dve else nc.scalar
                    if yeng is nc.vector:
                        nc.vector.tensor_copy(yk[:], ph[:, :, 0:TH])
                    else:
                        nc.scalar.copy(yk[:], ph[:, :, 0:TH])
                    pz = pz_pool.tile([BIN, H], f32, tag="pz")
                    nc.tensor.matmul(pz[:, 0:512], cb[:], yk[:, 0:512],
                                     start=True, stop=True)
                    nc.tensor.matmul(pz[:, 512:H], cb[:], yk[:, 512:H],
                                     start=True, stop=True)
                    assert btk_dma == 0
                    btp = bt_pool.tile([BIN, H], fp16, tag="btk")
                    nc.scalar.activation(btp[:], xt16[:],
                                         AF.Derivative_Erf, scale=10.0,
                                         bias=biases[0:BIN, k:k + 1])
                    btk = btp[:]
                    tk = t_pool.tile([BIN, H], fp16, tag="tk")
                    if tk_split:
                        zk = z_pool.tile([BIN, H], fp16, tag="zk")
                        if (k % k_img) < zk_act:
                            nc.scalar.mul(zk[:], pz[:], wgts[k])
                        else:
                            nc.vector.tensor_scalar_mul(zk[:], pz[:], wgts[k])
                        teng = nc.gpsimd if (k % k_img) < tk_gp else nc.vector
                        teng.tensor_tensor(tk[:], btk, zk[:], ALU.mult)
                    else:
                        nc.vector.scalar_tensor_tensor(tk[:], btk, wgts[k],
                                                       pz[:], ALU.mult,
                                                       ALU.mult)
                    acc_mm(id16[0:BIN, 0:BIN], tk)
                acc_mm(idn[:], xt16[:])
                of = of_pool.tile([BIN, H], fp16, tag="of")
                if of_act:
                    nc.scalar.copy(of[:], pacc[:])
                else:
                    nc.vector.tensor_copy(of[:], pacc[:])
                pb = ph_pool.tile([BIN, NT, 128], f32, tag="ph")
                for t in range(6):
                    nc.tensor.matmul(pb[0:128, t, 0:ncols],
                                     of[:, 128 * t:128 * (t + 1)],
                                     id16[0:BIN, D:D + ncols],
                                     start=True, stop=True)
                ob = of_pool.tile([128, 6, BW], f32, tag="ob")
                oeng = nc.scalar if (plane + band) % 2 == 0 else nc.vector
                if oeng is nc.scalar:
                    nc.scalar.copy(ob[:, :, 0:ncols], pb[:, 0:6, 0:ncols])
                else:
                    nc.vector.tensor_copy(ob[:, :, 0:ncols], pb[:, 0:6, 0:ncols])
                obase = plane * H * W + c0
                nc.sync.dma_start(
                    out=bass.AP(tensor=out, offset=obase,
                                ap=[[W, 128], [128 * W, 6], [1, ncols]]),
                    in_=ob[:, :, 0:ncols])

            def slab_unit(plane, s0):
                bands = list(range(s0, min(s0 + slab_nb, NB)))
                C0 = BW * s0
                wp = 128 + (len(bands) - 1) * BW
                base = plane * RPAD * CPAD + C0
                x16s = x16_pool.tile([128, NT, slabw], fp16, tag="x16")
                from contextlib import nullcontext
                with (tc.high_priority() if hoist_dma else nullcontext()):
                    nc.sync.dma_start(
                        out=x16s[:, :, 0:wp],
                        in_=bass.AP(tensor=xp, offset=base,
                                    ap=[[CPAD, 128], [TH * CPAD, NT], [1, wp]]))
                pks = []
                bkl = []
                for k in range(k_img):
                    bks = b_pool.tile([128, NT, slabw], fp16, tag="bk")
                    nc.scalar.activation(bks[:, :, 0:wp], x16s[:, :, 0:wp],
                                         AF.Derivative_Erf, scale=10.0,
                                         bias=biases[:, k:k + 1])
                    pk = p_pool.tile([128, NT, slabw], fp16, tag="pk")
                    peng = nc.gpsimd if (k % k_img) < p_gpsimd else nc.vector
                    peng.tensor_tensor(pk[:, :, 0:wp], bks[:, :, 0:wp],
                                       x16s[:, :, 0:wp], ALU.mult)
                    pks.append(pk)
                    bkl.append(bks)
                for b in bands:
                    band_tail(plane, b, x16s, pks, bkl, BW * (b - s0))

            def body(_iv=None):
                for plane in range(PLANES):
                    for s0 in range(0, NB, slab_nb):
                        slab_unit(plane, s0)

            if reps == 1:
                body()
            else:
                with tc.For_i(0, reps, 1) as _i:
                    body(_i)
    nc.compile()
    return nc


def _prepare_inputs(x):
    planes = np.ascontiguousarray(x.reshape(N_CORES, PLANES, H, W))
    bband, cband, ident16, idneg = _consts()
    in_maps = []
    for c in range(N_CORES):
        xpad = np.pad(planes[c], ((0, 0), (D, RPAD - H - D), (D, CPAD - W - D)),
                      mode="reflect").astype(np.float16)
        in_maps.append({"xp": np.ascontiguousarray(xpad),
                        "bband": bband, "cband": cband, "ident16": ident16,
                        "idneg": idneg})
    return in_maps


def _gather_outputs(results):
    outs = [results[c]["out"] for c in range(N_CORES)]
    return np.stack(outs).reshape(16, 3, H, W).astype(np.float32)


def kernel(x):
    from concourse.bass_utils import run_bass_kernel_spmd

    x = np.asarray(x, dtype=np.float32)
    if "nc" not in _CACHE:
        _CACHE["nc"] = build(reps=1)
    in_maps = _prepare_inputs(x)
    res = run_bass_kernel_spmd(_CACHE["nc"], in_maps,
                               core_ids=list(range(N_CORES)))
    return _gather_outputs(res.results)
